# revision 10
# baseline (speedup 1.0000x reference)
"""CopyRNN kernel for 8 NeuronCores (trn2) via Bass/Tile.

Sharding: the sequential encoder LSTM / decoder / attention are replicated on
all 8 cores (latency-bound), while all (V+OOV)-sized work — gen_proj matmul,
copy-score scatter, masks, exp — is sharded over the vocab axis (6272
rows/core).  The softmax denominator is combined with a tiny AllReduce and
each core emits its probability shard.

The (B, V+OOV, H) scatter-aggregate is never materialized: duplicate-token
encoder states are aggregated per source position with a 256x256 equality
matmul per batch row; each position's score is projected/tanh'd/dotted with
attn_out and scattered into the vocab shard via indirect DMA (duplicates
write identical values; out-of-shard positions go to a dump row).
"""

import numpy as np

import concourse.bass as bass
import concourse.bacc as bacc
import concourse.mybir as mybir
import concourse.tile as tile
from concourse.bass import IndirectOffsetOnAxis
from concourse.bass_utils import run_bass_kernel_spmd

F32 = mybir.dt.float32
I32 = mybir.dt.int32
AF = mybir.ActivationFunctionType
OP = mybir.AluOpType

B, L, V, OOV, E, H = 16, 256, 50000, 100, 128, 128
VOC = V + OOV            # 50100
NCORES = 8
NCH = 49                 # vocab chunks of 128 per core
VS = NCH * 128           # 6272 rows/core (8*6272 = 50176 >= 50100)
VPAD = 6400              # scatter table rows incl. dump row VS
LB = L * B               # 4096 (t-major columns)
TCH = 33                 # token gather chunks (33*128 >= 4096+16)
G4 = 64
NEG = -1.0e30

# chip gate order [f, i, o, g]; PyTorch order is [i, f, g, o]
PERM = [1, 0, 3, 2]


def _perm_gates(w):
    return np.concatenate([w[g * H:(g + 1) * H] for g in PERM], axis=0)


def build_program():
    nc = bacc.Bacc("TRN2", target_bir_lowering=False, num_devices=NCORES)

    def inp(name, shape, dtype=F32):
        return nc.dram_tensor(name, shape, dtype, kind="ExternalInput")

    emb_d = inp("emb", [V, E])
    tok_idx_d = inp("tok_idx", [128, TCH], I32)
    wih_d = inp("wih", [E, 4 * H])
    whh_d = inp("whh", [H, 4 * H])
    encb_d = inp("encb", [128, 4])
    dwih_d = inp("dwih", [E + H, 4 * H])
    decb_d = inp("decb", [128, 4])
    attw_d = inp("attw", [H, H])
    outw_d = inp("outw", [2 * H, H])
    outb_d = inp("outb", [128, 1])
    cpw_d = inp("cpw", [H, H])
    dst_d = inp("dst", [H, B])
    pm_d = inp("pm", [B, L])
    pad_d = inp("pad", [B, L])
    tokv_d = inp("tokv", [B, L])
    tokvT_d = inp("tokvT", [128, 2, B])
    bsel_d = inp("bsel", [B, 16 * 128])
    i16_d = inp("i16", [B, B])
    i128_d = inp("i128", [128, 128])
    gsh_d = inp("gsh", [VS, H])
    vmask_d = inp("vmask", [128, NCH, B])
    tloc_d = inp("tloc", [128, 2, B], I32)   # flat (= row*16 + b), dump ok

    prob_o = nc.dram_tensor("prob", [VS, B], F32, kind="ExternalOutput")
    enc_o = nc.dram_tensor("enc", [LB, H], F32, kind="ExternalOutput")
    attn_o = nc.dram_tensor("attn", [H, B], F32, kind="ExternalOutput")

    cs_dram = nc.dram_tensor("cs_scratch", [VPAD, B], F32)

    with tile.TileContext(nc) as tc:
        with (
            tc.tile_pool(name="persist", bufs=1) as pp,
            tc.tile_pool(name="gen", bufs=1) as gp,
            tc.tile_pool(name="work", bufs=2) as wp,
            tc.tile_pool(name="dram", bufs=1, space="DRAM") as dp,
        ):
            # ---------- small tensor loads ----------
            def load(pool, d, shape, dtype=F32):
                t = pool.tile(shape, dtype, tag=d.name + "_sb")
                nc.sync.dma_start(t[:], d[:])
                return t

            tok_idx = load(pp, tok_idx_d, [128, TCH], I32)
            wih = load(pp, wih_d, [E, 4 * H])
            whh = load(pp, whh_d, [H, 4 * H])
            encb = load(pp, encb_d, [128, 4])
            dwih = pp.tile([128, 2, 4 * H], F32, tag="dwih_sb")
            nc.sync.dma_start(dwih[:, 0, :], dwih_d[0:128, :])
            nc.sync.dma_start(dwih[:, 1, :], dwih_d[128:256, :])
            decb = load(pp, decb_d, [128, 4])
            attw = load(pp, attw_d, [H, H])
            outw = pp.tile([128, 2, H], F32, tag="outw_sb")
            nc.sync.dma_start(outw[:, 0, :], outw_d[0:128, :])
            nc.sync.dma_start(outw[:, 1, :], outw_d[128:256, :])
            outb = load(pp, outb_d, [128, 1])
            cpw = load(pp, cpw_d, [H, H])
            dst = load(pp, dst_d, [H, B])
            pm = load(pp, pm_d, [B, L])
            pad = load(pp, pad_d, [B, L])
            tokv = load(pp, tokv_d, [B, L])
            tokvT = load(pp, tokvT_d, [128, 2, B])
            bsel = load(pp, bsel_d, [B, 16 * 128])
            i16 = load(pp, i16_d, [B, B])
            i128 = load(pp, i128_d, [128, 128])
            vmask = load(gp, vmask_d, [128, NCH, B])
            tloc = load(pp, tloc_d, [128, 2, B], I32)

            ones128 = pp.tile([128, 1], F32, tag="ones128")
            nc.vector.memset(ones128[:], 1.0)
            ones1x = pp.tile([1, 128], F32, tag="ones1x")
            nc.vector.memset(ones1x[:], 1.0)
            ones16 = pp.tile([B, 128], F32, tag="ones16")
            nc.vector.memset(ones16[:], 1.0)

            gT = gp.tile([128, NCH, 128], F32, tag="gT")
            encT = pp.tile([H, LB], F32, tag="encT")
            semb = pp.tile([E, B], F32, tag="semb")
            attn = pp.tile([H, B], F32, tag="attn")

            # ---------- phase 1: gather + xg + LSTM + gT transposes ----------
            with (
                tc.tile_pool(name="xgp", bufs=1) as xp,
                tc.tile_pool(name="ps1", bufs=2, space="PSUM") as ps1,
            ):
                xg = xp.tile([128, L, G4], F32, tag="xg")
                for c8 in range(9):
                    embc = wp.tile([E, 512], F32, tag="embc")
                    nchunks = 4 if c8 < 8 else 1
                    for cc in range(nchunks):
                        c = c8 * 4 + cc
                        eg = wp.tile([128, E], F32, tag="embg")
                        nc.gpsimd.indirect_dma_start(
                            out=eg[:], out_offset=None, in_=emb_d[:],
                            in_offset=IndirectOffsetOnAxis(
                                ap=tok_idx[:, c:c + 1], axis=0),
                        )
                        ps = ps1.tile([128, 128], F32, tag="psT")
                        nc.tensor.transpose(ps[:], eg[:], i128[:])
                        nc.scalar.copy(embc[:, cc * 128:(cc + 1) * 128], ps[:])
                    if c8 == 8:
                        nc.vector.tensor_copy(semb[:], embc[:, 0:B])
                        break
                    for g in range(4):
                        ps = ps1.tile([128, 512], F32, tag="ps512")
                        nc.tensor.matmul(
                            ps[:], wih[:, g * 128:(g + 1) * 128], embc[:],
                            start=True, stop=True)
                        nc.scalar.activation(
                            xg[:, c8 * 32:(c8 + 1) * 32, g * 16:(g + 1) * 16],
                            ps[:].rearrange("p (t b) -> p t b", b=16),
                            AF.Identity, bias=encb[:, g:g + 1])

                h0 = pp.tile([H, B], F32, tag="h0")
                nc.vector.memset(h0[:], 0.0)
                cbuf = pp.tile([H, G4], F32, tag="cbuf")  # [c0|tg0|c1|tg1]
                nc.vector.memset(cbuf[:], 0.0)

                for t in range(L):
                    e = t % 2
                    hprev = h0[:] if t == 0 else encT[:, (t - 1) * B:t * B]
                    ps = ps1.tile([128, G4], F32, tag="ps64")
                    nc.tensor.matmul(ps[:], i128[:], xg[:, t, :],
                                     start=True, stop=False)
                    for g in range(4):
                        nc.tensor.matmul(
                            ps[:, g * 16:(g + 1) * 16],
                            whh[:, g * 128:(g + 1) * 128], hprev,
                            start=False, stop=(g == 3))
                    sig = wp.tile([H, 48], F32, tag="sig")
                    nc.scalar.activation(sig[:], ps[:, 0:48], AF.Sigmoid)
                    tgcol = 16 if e == 0 else 48
                    ccol = 0 if e == 0 else 32
                    ncol = 32 if e == 0 else 0
                    nc.scalar.activation(cbuf[:, tgcol:tgcol + 16],
                                         ps[:, 48:64], AF.Tanh)
                    prod = wp.tile([H, 32], F32, tag="prod")
                    nc.vector.tensor_tensor(
                        out=prod[:].rearrange("p (k j) -> p k j", j=2),
                        in0=sig[:, 0:32].rearrange("p (j k) -> p k j", j=2),
                        in1=cbuf[:, ccol:ccol + 32].rearrange(
                            "p (j k) -> p k j", j=2),
                        op=OP.mult)
                    nc.vector.tensor_reduce(
                        out=cbuf[:, ncol:ncol + 16],
                        in_=prod[:].rearrange("p (k j) -> p k j", j=2),
                        axis=mybir.AxisListType.X, op=OP.add)
                    tcl = wp.tile([H, B], F32, tag="tc")
                    nc.scalar.activation(tcl[:], cbuf[:, ncol:ncol + 16],
                                         AF.Tanh)
                    nc.vector.tensor_tensor(
                        out=encT[:, t * B:(t + 1) * B],
                        in0=sig[:, 32:48], in1=tcl[:], op=OP.mult)
                    if t % 5 == 0 and t // 5 < NCH:
                        c = t // 5
                        gch = wp.tile([128, H], F32, tag="gch")
                        nc.sync.dma_start(gch[:], gsh_d[c * 128:(c + 1) * 128, :])
                        psg = ps1.tile([128, 128], F32, tag="psT")
                        nc.tensor.transpose(psg[:], gch[:], i128[:])
                        nc.scalar.copy(gT[:, c, :], psg[:])

            # ---------- phase 2: attention blocks ----------
            with tc.tile_pool(name="ps2", bufs=2, space="PSUM") as ps2:
                i16bc = i16[:].rearrange("p (o b) -> p o b", o=1)

                def weighted_sum(row_sb, out_hb, score_sink=None, proj=None):
                    """out_hb (H,B) = sum_t row[b,t] * encT[:, t*16+b].

                    If proj is given: first y = tanh(proj @ (rep * encT)),
                    score_sink (16,L) gets diag(dst^T @ y) instead, and
                    out_hb is unused.
                    """
                    parts = wp.tile([128, 8, B], F32, tag="parts")
                    for j in range(8):
                        rt = wp.tile([B, 512], F32, tag="rt")
                        nc.vector.tensor_tensor(
                            out=rt[:].rearrange("p (t b) -> p t b", b=16),
                            in0=row_sb[:, j * 32:(j + 1) * 32].rearrange(
                                "p (t o) -> p t o", o=1).broadcast_to([B, 32, B]),
                            in1=i16bc.broadcast_to([B, 32, B]),
                            op=OP.mult)
                        ps = ps2.tile([128, 512], F32, tag="ps512b")
                        nc.tensor.matmul(ps[:], ones16[:], rt[:],
                                         start=True, stop=True)
                        mch = wp.tile([128, 512], F32, tag="mch")
                        nc.vector.tensor_tensor(
                            out=mch[:], in0=encT[:, j * 512:(j + 1) * 512],
                            in1=ps[:], op=OP.mult)
                        if proj is None:
                            nc.vector.tensor_reduce(
                                out=parts[:, j, :],
                                in_=mch[:].rearrange("p (t b) -> p b t", b=16),
                                axis=mybir.AxisListType.X, op=OP.add)
                        else:
                            ps2b = ps2.tile([128, 512], F32, tag="ps512b")
                            nc.tensor.matmul(ps2b[:], proj, mch[:],
                                             start=True, stop=True)
                            ych = wp.tile([128, 512], F32, tag="mch")
                            nc.scalar.activation(ych[:], ps2b[:], AF.Tanh)
                            pss = ps2.tile([B, 512], F32, tag="psS")
                            nc.tensor.matmul(pss[:], dst[:], ych[:],
                                             start=True, stop=True)
                            scm = wp.tile([B, 512], F32, tag="rt")
                            nc.vector.tensor_tensor(
                                out=scm[:].rearrange("p (t b) -> p t b", b=16),
                                in0=pss[:].rearrange("p (t b) -> p t b", b=16),
                                in1=i16bc.broadcast_to([B, 32, B]),
                                op=OP.mult)
                            nc.vector.tensor_reduce(
                                out=score_sink[:, j * 32:(j + 1) * 32],
                                in_=scm[:].rearrange("p (t b) -> p t b", b=16),
                                axis=mybir.AxisListType.X, op=OP.add)
                    if proj is None:
                        nc.vector.tensor_reduce(
                            out=out_hb,
                            in_=parts[:].rearrange("p j b -> p b j"),
                            axis=mybir.AxisListType.X, op=OP.add)

                # selective-read scores
                cwsc = pp.tile([B, L], F32, tag="cwsc")
                weighted_sum(pm, None, score_sink=cwsc, proj=cpw[:])
                cwexp = pp.tile([B, L], F32, tag="cwexp")
                cwsum = pp.tile([B, 1], F32, tag="cwsum")
                nc.scalar.activation(cwexp[:], cwsc[:], AF.Exp,
                                     accum_out=cwsum[:])
                cwrec = pp.tile([B, 1], F32, tag="cwrec")
                nc.vector.reciprocal(cwrec[:], cwsum[:])
                cwn = pp.tile([B, L], F32, tag="cwn")
                nc.vector.tensor_scalar_mul(cwn[:], cwexp[:], cwrec[:, 0:1])
                cstate = pp.tile([H, B], F32, tag="cstate")
                weighted_sum(cwn, cstate[:])

                # decoder single LSTM step (c0 = 0 -> f gate unused)
                dps = ps2.tile([128, G4], F32, tag="ps64b")
                for g in (1, 2, 3):
                    nc.tensor.matmul(dps[:, g * 16:(g + 1) * 16],
                                     dwih[:, 0, g * 128:(g + 1) * 128],
                                     semb[:], start=True, stop=False)
                    nc.tensor.matmul(dps[:, g * 16:(g + 1) * 16],
                                     dwih[:, 1, g * 128:(g + 1) * 128],
                                     cstate[:], start=False, stop=True)
                dsi = wp.tile([H, B], F32, tag="dsi")
                dso = wp.tile([H, B], F32, tag="dso")
                dtg = wp.tile([H, B], F32, tag="dtg")
                nc.scalar.activation(dsi[:], dps[:, 16:32], AF.Sigmoid,
                                     bias=decb[:, 1:2])
                nc.scalar.activation(dso[:], dps[:, 32:48], AF.Sigmoid,
                                     bias=decb[:, 2:3])
                nc.scalar.activation(dtg[:], dps[:, 48:64], AF.Tanh,
                                     bias=decb[:, 3:4])
                dc = wp.tile([H, B], F32, tag="dc")
                nc.vector.tensor_tensor(out=dc[:], in0=dsi[:], in1=dtg[:],
                                        op=OP.mult)
                dtc = wp.tile([H, B], F32, tag="dtc")
                nc.scalar.activation(dtc[:], dc[:], AF.Tanh)
                rnn = pp.tile([H, B], F32, tag="rnn")
                nc.vector.tensor_tensor(out=rnn[:], in0=dso[:], in1=dtc[:],
                                        op=OP.mult)

                # attention scores (chunked diag of q^T @ encT)
                qps = ps2.tile([H, B], F32, tag="psQ")
                nc.tensor.matmul(qps[:], attw[:], rnn[:], start=True,
                                 stop=True)
                q = wp.tile([H, B], F32, tag="q")
                nc.vector.tensor_copy(q[:], qps[:])
                ssc = pp.tile([B, L], F32, tag="ssc")
                for j in range(8):
                    pss = ps2.tile([B, 512], F32, tag="psS")
                    nc.tensor.matmul(pss[:], q[:],
                                     encT[:, j * 512:(j + 1) * 512],
                                     start=True, stop=True)
                    scm = wp.tile([B, 512], F32, tag="rt")
                    nc.vector.tensor_tensor(
                        out=scm[:].rearrange("p (t b) -> p t b", b=16),
                        in0=pss[:].rearrange("p (t b) -> p t b", b=16),
                        in1=i16bc.broadcast_to([B, 32, B]), op=OP.mult)
                    nc.vector.tensor_reduce(
                        out=ssc[:, j * 32:(j + 1) * 32],
                        in_=scm[:].rearrange("p (t b) -> p t b", b=16),
                        axis=mybir.AxisListType.X, op=OP.add)
                nc.vector.tensor_tensor(out=ssc[:], in0=ssc[:], in1=pad[:],
                                        op=OP.add)
                sexp = pp.tile([B, L], F32, tag="sexp")
                ssum = pp.tile([B, 1], F32, tag="ssum")
                nc.scalar.activation(sexp[:], ssc[:], AF.Exp,
                                     accum_out=ssum[:])
                srec = pp.tile([B, 1], F32, tag="srec")
                nc.vector.reciprocal(srec[:], ssum[:])
                aw = pp.tile([B, L], F32, tag="aw")
                nc.vector.tensor_scalar_mul(aw[:], sexp[:], srec[:, 0:1])
                ctx = pp.tile([H, B], F32, tag="ctx")
                weighted_sum(aw, ctx[:])
                aps_ = ps2.tile([H, B], F32, tag="psQ")
                nc.tensor.matmul(aps_[:], outw[:, 0, :], rnn[:],
                                 start=True, stop=False)
                nc.tensor.matmul(aps_[:], outw[:, 1, :], ctx[:],
                                 start=False, stop=True)
                nc.scalar.activation(attn[:], aps_[:], AF.Tanh,
                                     bias=outb[:, 0:1])
                nc.sync.dma_start(attn_o[:], attn[:])

            # ---------- phase 3: enc_out, copy scores, gen, softmax ----------
            with (
                tc.tile_pool(name="ps3", bufs=2, space="PSUM") as ps3,
                tc.tile_pool(name="psR", bufs=1, space="PSUM") as psR,
            ):
                encBL = pp.tile([128, 32, H], F32, tag="encBL")
                encv = encT[:].rearrange("p (h2 t b2) -> p h2 t b2",
                                         h2=2, b2=16)
                for b in range(B):
                    for hf in range(2):
                        ps = ps3.tile([128, 128], F32, tag="psT3")
                        nc.tensor.transpose(ps[:], encv[:, hf, :, b], i128[:])
                        nc.scalar.copy(encBL[:, b * 2 + hf, :], ps[:])
                nc.sync.dma_start(
                    enc_o[:].rearrange("(c p) h -> p c h", p=128), encBL[:])

                csc0 = pp.tile([B, L], F32, tag="csc0")
                csc1 = pp.tile([B, L], F32, tag="csc1")
                nc.vector.memset(csc0[:], 0.0)
                for b in range(B):
                    rps = ps3.tile([128, L], F32, tag="ps256")
                    nc.tensor.matmul(rps[:], bsel[:, b * 128:(b + 1) * 128],
                                     tokv[:], start=True, stop=True)
                    msb = wp.tile([128, 2, L], F32, tag="msb")
                    for hf in range(2):
                        nc.vector.tensor_scalar(
                            out=msb[:, hf, :], in0=rps[:],
                            scalar1=tokvT[:, hf, b:b + 1], scalar2=None,
                            op0=OP.is_equal)
                    agp = ps3.tile([128, L], F32, tag="ps256")
                    for hf in range(2):
                        nc.tensor.matmul(agp[:], encBL[:, b * 2 + hf, :],
                                         msb[:, hf, :],
                                         start=(hf == 0), stop=(hf == 1))
                    ags = wp.tile([128, L], F32, tag="ags")
                    nc.vector.tensor_copy(ags[:], agp[:])
                    pjp = ps3.tile([128, L], F32, tag="ps256")
                    nc.tensor.matmul(pjp[:], cpw[:], ags[:], start=True,
                                     stop=True)
                    cwt = wp.tile([128, L], F32, tag="cwt")
                    nc.scalar.activation(cwt[:], pjp[:], AF.Tanh)
                    s16 = ps3.tile([B, L], F32, tag="psM")
                    nc.tensor.matmul(s16[:], attn[:], cwt[:], start=True,
                                     stop=True)
                    dst_t, src_t = (csc1, csc0) if b % 2 == 0 else (csc0, csc1)
                    nc.vector.scalar_tensor_tensor(
                        out=dst_t[:], in0=s16[:], scalar=i16[:, b:b + 1],
                        in1=src_t[:], op0=OP.mult, op1=OP.add)
                csc = csc0

                zer = pp.tile([128, 50, B], F32, tag="zer")
                nc.vector.memset(zer[:], 0.0)
                nc.sync.dma_start(
                    cs_dram[:].rearrange("(c p) b -> p c b", p=128), zer[:])
                scT = pp.tile([128, 2, B], F32, tag="scT")
                for hf in range(2):
                    ps = ps3.tile([128, B], F32, tag="psM")
                    nc.tensor.transpose(ps[:],
                                        csc[:, hf * 128:(hf + 1) * 128],
                                        i16[:])
                    nc.vector.tensor_copy(scT[:, hf, :], ps[:])
                cs_flat = cs_dram[:].rearrange("r (b o) -> (r b) o", o=1)
                for b in range(B):
                    for hf in range(2):
                        nc.gpsimd.indirect_dma_start(
                            out=cs_flat,
                            out_offset=IndirectOffsetOnAxis(
                                ap=tloc[:, hf, b:b + 1], axis=0),
                            in_=scT[:, hf, b:b + 1],
                            in_offset=None,
                        )
                csT = gp.tile([128, 50, B], F32, tag="csT")
                nc.sync.dma_start(
                    csT[:], cs_dram[:].rearrange("(c p) b -> p c b", p=128))

                glog = gp.tile([128, NCH, B], F32, tag="glog")
                for c in range(NCH):
                    ps = ps3.tile([128, B], F32, tag="psM")
                    nc.tensor.matmul(ps[:], gT[:, c, :], attn[:],
                                     start=True, stop=True)
                    nc.scalar.copy(glog[:, c, :], ps[:])
                tot = gp.tile([128, NCH * B], F32, tag="tot")
                nc.vector.tensor_tensor(
                    out=tot[:].rearrange("p (c b) -> p c b", b=16),
                    in0=glog[:], in1=csT[:, 0:NCH, :], op=OP.add)
                nc.vector.tensor_tensor(
                    out=tot[:].rearrange("p (c b) -> p c b", b=16),
                    in0=tot[:].rearrange("p (c b) -> p c b", b=16),
                    in1=vmask[:], op=OP.add)
                expT = gp.tile([128, NCH * B], F32, tag="expT")
                nc.scalar.activation(expT[:], tot[:], AF.Exp)

                sps = psR.tile([1, NCH * B], F32, tag="psRow")
                nc.tensor.matmul(sps[:, 0:512], ones128[:], expT[:, 0:512],
                                 start=True, stop=True)
                nc.tensor.matmul(sps[:, 512:NCH * B], ones128[:],
                                 expT[:, 512:NCH * B], start=True, stop=True)
                spart = pp.tile([1, B], F32, tag="spart")
                nc.vector.tensor_reduce(
                    out=spart[:],
                    in_=sps[:].rearrange("p (c b) -> p b c", b=16),
                    axis=mybir.AxisListType.X, op=OP.add)
                inb = dp.tile([1, B], F32)
                outbb = dp.tile([1, B], F32)
                nc.gpsimd.dma_start(inb[:], spart[:])
                nc.gpsimd.collective_compute(
                    "AllReduce", OP.add,
                    replica_groups=[list(range(NCORES))],
                    ins=[inb.opt()], outs=[outbb.opt()])
                stot = pp.tile([1, B], F32, tag="stot")
                nc.sync.dma_start(stot[:], outbb[:])
                srz = pp.tile([1, B], F32, tag="srz")
                nc.vector.reciprocal(srz[:], stot[:])
                rzp = ps3.tile([128, B], F32, tag="psM")
                nc.tensor.matmul(rzp[:], ones1x[:], srz[:], start=True,
                                 stop=True)
                rz = pp.tile([128, B], F32, tag="rz")
                nc.vector.tensor_copy(rz[:], rzp[:])
                prob = gp.tile([128, NCH * B], F32, tag="probT")
                nc.vector.tensor_tensor(
                    out=prob[:].rearrange("p (c b) -> p c b", b=16),
                    in0=expT[:].rearrange("p (c b) -> p c b", b=16),
                    in1=rz[:].rearrange("p (o b) -> p o b", o=1).broadcast_to(
                        [128, NCH, B]),
                    op=OP.mult)
                nc.sync.dma_start(
                    prob_o[:].rearrange("(c p) b -> p c b", p=128), prob[:])

    nc.finalize()
    return nc


_PROG = None
TRACE = False
LAST = None
LAST_INMAPS = None


def kernel(**inputs):
    global _PROG
    inp = {k: np.asarray(v) for k, v in inputs.items()}
    src = inp["src_tokens"].astype(np.int64)
    prev = inp["prev_output_tokens"].astype(np.int64)
    tokv = inp["src_tokens_with_oov"].astype(np.int64)
    oovc = inp["oov_counts"].astype(np.int64)

    toks = np.zeros(TCH * 128, dtype=np.int32)
    toks[:LB] = src.T.reshape(-1)          # t-major
    toks[LB:LB + B] = prev[:, 0]
    tok_idx = np.ascontiguousarray(toks.reshape(TCH, 128).T)

    wih = np.ascontiguousarray(_perm_gates(inp["enc_W_ih"].astype(np.float32)).T)
    whh = np.ascontiguousarray(_perm_gates(inp["enc_W_hh"].astype(np.float32)).T)
    encb = np.ascontiguousarray(
        _perm_gates(inp["enc_b"].astype(np.float32).reshape(4 * H, 1))
        .reshape(4, H).T)
    dwih = np.ascontiguousarray(_perm_gates(inp["dec_W_ih"].astype(np.float32)).T)
    decb = np.ascontiguousarray(
        _perm_gates(inp["dec_b"].astype(np.float32).reshape(4 * H, 1))
        .reshape(4, H).T)

    pm = (prev[:, 0:1] == src).astype(np.float32)
    pad = np.where(src == 0, NEG, 0.0).astype(np.float32)
    tokvf = tokv.astype(np.float32)
    tokvT = np.ascontiguousarray(
        np.transpose(tokvf.reshape(B, 2, 128), (2, 1, 0)))

    bsel = np.zeros((B, 16 * 128), dtype=np.float32)
    for b in range(B):
        bsel[b, b * 128:(b + 1) * 128] = 1.0
    i16 = np.eye(B, dtype=np.float32)
    i128 = np.eye(128, dtype=np.float32)

    common = dict(
        emb=inp["embedding"].astype(np.float32),
        tok_idx=tok_idx, wih=wih, whh=whh, encb=encb, dwih=dwih, decb=decb,
        attw=np.ascontiguousarray(inp["attn_W"].astype(np.float32).T),
        outw=np.ascontiguousarray(inp["out_W"].astype(np.float32).T),
        outb=inp["out_b"].astype(np.float32).reshape(H, 1),
        cpw=np.ascontiguousarray(inp["copy_W"].astype(np.float32).T),
        dst=np.ascontiguousarray(inp["decoder_state"].astype(np.float32).T),
        pm=pm, pad=pad, tokv=tokvf, tokvT=tokvT,
        bsel=bsel, i16=i16, i128=i128,
    )

    genw = inp["gen_W"].astype(np.float32)
    in_maps = []
    for k in range(NCORES):
        base = k * VS
        gsh = np.zeros((VS, H), dtype=np.float32)
        hi = min(base + VS, VOC)
        if base < VOC:
            gsh[0:hi - base] = genw[base:hi]
        gl = np.arange(base, base + VS)
        vm = np.zeros((B, VS), dtype=np.float32)
        vm = np.where(gl[None, :] >= (V + oovc)[:, None], NEG, vm)
        vm = np.where(gl[None, :] == 1, NEG, vm)
        vm = np.where(gl[None, :] >= VOC, NEG, vm)
        vmT = np.ascontiguousarray(
            np.transpose(vm.astype(np.float32).reshape(B, NCH, 128),
                         (2, 1, 0)))
        tl = tokv - base                                  # (B, L)
        tl = np.where((tl < 0) | (tl >= VS), VS, tl)
        tlB = (tl * B + np.arange(B)[:, None]).astype(np.int32)
        tlT = np.ascontiguousarray(
            np.transpose(tlB.reshape(B, 2, 128), (2, 1, 0)))
        m = dict(common)
        m.update(gsh=gsh, vmask=vmT, tloc=tlT)
        in_maps.append(m)

    global LAST, LAST_INMAPS
    LAST_INMAPS = in_maps
    if _PROG is None:
        _PROG = build_program()
    res = run_bass_kernel_spmd(_PROG, in_maps, core_ids=list(range(NCORES)),
                               trace=TRACE)
    LAST = res

    probs = np.concatenate([res.results[k]["prob"] for k in range(NCORES)],
                           axis=0)
    total_prob = np.ascontiguousarray(probs[:VOC].T)
    enc_out = res.results[0]["enc"].reshape(B, L, H)
    attn_out = np.ascontiguousarray(res.results[0]["attn"].T)
    return (total_prob.astype(np.float32), enc_out.astype(np.float32),
            attn_out.astype(np.float32))
